# revision 53
# baseline (speedup 1.0000x reference)
"""Online Normalization (forward) on 8 Trainium2 NeuronCores.

Reference semantics (per batch sample t, stats per channel over H*W):
    out_t = (x_t - s_mu_{t-1}) / sqrt(s_var_{t-1} + eps)
    mu_t  = mean(x_t);  var_t = mean(x_t^2) - mu_t^2
    s_mu_t  = a*s_mu_{t-1}  + (1-a)*mu_t
    s_var_t = a*s_var_{t-1} + (1-a)*var_t + a*(1-a)*(mu_t - s_mu_{t-1})^2

The EMA recurrence is linear, so instead of a sequential scan over the batch
axis we compute per-sample batch stats in parallel and apply the recurrence
as small matmuls on the tensor engine:
    s_mu_{t-1}  = a^t mu0  + sum_i W[i,t] * mu_i,   W[i,t] = (1-a) a^{t-1-i}, i<t
    s_var_{t-1} = a^t var0 + sum_i W[i,t] * f_i,    f_i = var_i + a*d_i^2,
                                                    d_i = mu_i - s_mu_{i-1}

v22: x and out are staged in bf16 (tolerance is 2e-2; bf16 staging costs
~2e-3), halving DMA traffic; the const blob loads FIRST so the scan can
start as soon as the first samples land.  The DVE runs a pure bn_stats
"spine" (mean+var per sample in one pass) with NOTHING else queued on it,
so it streams gap-free; the scan chain runs entirely off-spine (PSUM->SBUF
copies and 1/sqrt on ACT, init terms accumulated on the PE, f-recurrence
small ops on GPSIMD, transposes on the PE).  Early samples normalize on
ACT/GPSIMD while the spine runs; the whole tail (t>=16) is ONE fused scan
group whose normalizes stream on DVE right after the spine drains, with
their output DMAs on the SP queue (a GPSIMD teardown drain would stall the
SWDGE queue).  The tail group also skips bn_aggr entirely -- its combine
matmuls consume the raw bn_stats fields (means/cnt*var) with 1/16 and
1/4096 masks accumulated in PSUM -- and its recurrence small-ops run on the
now-idle DVE instead of GPSIMD, shortening the one off-spine chain.

Sharding: channels C=256 split across 8 cores (32 each) -- every channel's
recurrence is independent. Per core the shard sits resident in SBUF as
[128 partitions, 32 t, 1024 f] bf16, partition p = q*32 + c (q = one of 4
spatial blocks, c = channel).
"""

import os
import sys

import numpy as np

sys.path.insert(0, "/opt/trn_rl_repo")

B = 32          # batch (sequential scan axis)
H = 64
W_SP = 64
C = 256
NCORES = 8
CS = C // NCORES    # 32 channels per core
Q = 4               # spatial blocks per sample
F = (H * W_SP) // Q  # 1024 elements per block
P = 128             # partitions (Q*CS)
AFWD = 0.999
EPS = 1e-5
# scan groups (samples per scan matmul batch); tapered head for early output
GROUPS = [2, 6, 8, 16]
assert sum(GROUPS) == B

# engine assignment by sample index.  ACT_STAT samples get their stats from
# two ACT accumulate passes (Identity -> sum, Square -> sumsq) instead of DVE
# bn_stats, offloading the DVE stat stream; they sit at the tails of groups
# 1-3 so the ACT work overlaps DVE's bn_stats of the same group's head.
ACT_STAT = set()                            # pure bn_stats spine on DVE
NORM_GPS = {3, 7, 11, 15}                   # one GPSIMD norm per odd pair
NORM_DVE = set(range(16, 32))               # tail norms: DVE epilogue after spine

# const blob free-axis layout (f32, [128, CBLOB])
OFF_W = 0            # w     [B, B]     rows 0..31
OFF_MASK = OFF_W + B      # mask  [P, CS]
OFF_BMASK = OFF_MASK + CS  # bmask [CS, P]  rows 0..31
OFF_INIT = OFF_BMASK + P  # init  [CS, 2B] rows 0..31
OFF_EYE = OFF_INIT + 2 * B  # eye   [CS, CS] rows 0..31
CBLOB = OFF_EYE + CS

LAST_EXEC_NS = None
LAST_RESULTS = None
_COMPILED = {}


def _ensure_ntff_hook():
    """The axon boot degrades silently when ``antenv.axon_hooks`` is missing;
    provide the module + the ctypes-based NRT-profile hook ourselves so
    ``run_bass_kernel_spmd(trace=True)`` can capture NTFF profiles."""
    try:
        from antenv.axon_hooks import get_axon_ntff_profile_hook  # noqa: F401

        return
    except ImportError:
        pass

    import contextlib
    import ctypes
    import types

    so_path = "/opt/axon/libaxon_pjrt.so"
    state = {"hook": None}

    mod = types.ModuleType("antenv.axon_hooks")

    def set_axon_ntff_profile_hook(h):
        state["hook"] = h

    def get_axon_ntff_profile_hook():
        return state["hook"]

    mod.set_axon_ntff_profile_hook = set_axon_ntff_profile_hook
    mod.get_axon_ntff_profile_hook = get_axon_ntff_profile_hook
    import antenv

    antenv.axon_hooks = mod
    sys.modules["antenv.axon_hooks"] = mod

    if not os.path.exists(so_path):
        return
    lib = ctypes.CDLL(so_path)
    if not hasattr(lib, "axon_start_nrt_profile"):
        return
    lib.axon_start_nrt_profile.argtypes = [
        ctypes.POINTER(ctypes.c_int64),
        ctypes.c_size_t,
    ]
    lib.axon_start_nrt_profile.restype = ctypes.c_int64
    lib.axon_stop_nrt_profile.argtypes = [ctypes.c_char_p]
    lib.axon_stop_nrt_profile.restype = ctypes.c_int64

    @contextlib.contextmanager
    def _hook(output_dir, device_ids):
        import jax

        jax.devices()
        if device_ids:
            ids = (ctypes.c_int64 * len(device_ids))(*device_ids)
            rc = lib.axon_start_nrt_profile(ids, len(device_ids))
        else:
            rc = lib.axon_start_nrt_profile(None, 0)
        if rc != 0:
            raise RuntimeError(f"axon_start_nrt_profile rc={rc}")
        try:
            yield
        finally:
            n = lib.axon_stop_nrt_profile(str(output_dir).encode())
            print(f"profile: {n} file(s) written to {output_dir}", file=sys.stderr)

    state["hook"] = _hook


def _build_bass():
    from contextlib import ExitStack

    import concourse.bacc as bacc
    import concourse.tile as tile
    from concourse import mybir

    DT = mybir.dt.float32
    BF = mybir.dt.bfloat16
    Alu = mybir.AluOpType
    Act = mybir.ActivationFunctionType
    Ax = mybir.AxisListType

    nc = bacc.Bacc(
        "TRN2", target_bir_lowering=False, debug=False, num_devices=NCORES
    )
    x_h = nc.declare_dram_parameter("x", [P, B, F], BF, isOutput=False)
    blob_h = nc.declare_dram_parameter("blob", [P, CBLOB], DT, isOutput=False)
    maskb_h = nc.declare_dram_parameter("maskb", [P, 3 * CS], BF, isOutput=False)
    out_h = nc.declare_dram_parameter("out", [P, B, F], BF, isOutput=True)

    LMAX = max(GROUPS)
    NPAIR = B // 2

    with tile.TileContext(nc) as tc, ExitStack() as ctx:
        consts = ctx.enter_context(tc.tile_pool(name="consts", bufs=1))
        xpool = ctx.enter_context(tc.tile_pool(name="xp", bufs=1))
        sqpool = ctx.enter_context(tc.tile_pool(name="sqp", bufs=3))
        small = ctx.enter_context(tc.tile_pool(name="small", bufs=1))
        gpool = ctx.enter_context(tc.tile_pool(name="gp", bufs=3))
        psum = ctx.enter_context(tc.tile_pool(name="ps", bufs=2, space="PSUM"))

        blob = consts.tile([P, CBLOB], DT)
        sb_maskb3 = consts.tile([P, 3 * CS], BF)  # /Q | /16 | /4096 in bf16
        sb_maskb = sb_maskb3[:, 0:CS]
        sb_mask16 = sb_maskb3[:, CS : 2 * CS]
        sb_mask4k = sb_maskb3[:, 2 * CS : 3 * CS]
        sb_w = blob[0:B, OFF_W : OFF_W + B]          # [B, B]
        sb_mask = blob[:, OFF_MASK : OFF_MASK + CS]  # [P, CS]  (p%CS==c)/(Q*F)
        sb_bmask = blob[0:CS, OFF_BMASK : OFF_BMASK + P]  # [CS, P]
        sb_init = blob[0:CS, OFF_INIT : OFF_INIT + 2 * B]  # [CS, 2B]
        sb_eye = blob[0:CS, OFF_EYE : OFF_EYE + CS]  # [CS, CS] identity

        xbig = xpool.tile([P, B, F], BF)        # resident shard, 64 KiB/partition
        # ---- all input DMAs up front (SP queue); sample 0 triggers before
        # the const blobs (consts aren't needed until the first scan ~+6us)
        nc.sync.dma_start(out=xbig[:, 0, 0 : F // 2], in_=x_h[:, 0, 0 : F // 2])
        nc.sync.dma_start(out=xbig[:, 0, F // 2 : F], in_=x_h[:, 0, F // 2 : F])
        nc.sync.dma_start(out=blob, in_=blob_h[:, :])
        nc.sync.dma_start(out=sb_maskb3, in_=maskb_h[:, :])
        nc.sync.dma_start(out=xbig[:, 1:2, :], in_=x_h[:, 1:2, :])
        for k in range(1, NPAIR):
            nc.sync.dma_start(
                out=xbig[:, 2 * k : 2 * k + 2, :], in_=x_h[:, 2 * k : 2 * k + 2, :]
            )

        sb_eps = consts.tile([CS, 1], DT)
        nc.vector.memset(sb_eps, EPS)

        st6 = small.tile([P, B, 2, 6], BF)      # bn_stats out, 2 chunks/sample
        mv = small.tile([P, 2, B], BF)          # [,0,t]=mean  [,1,t]=var (pre-fix)
        s2 = small.tile([P, 2, B], DT)          # ACT_STAT raw sums / sumsq
        # Scan state in ct layout ([channel, t]): per-group writes slice the
        # FREE axis (partition slices must start at 0 on compute engines).
        mu_msq = small.tile([CS, 2 * B], DT)    # cols t: mu_ct; cols B+t: msq_ct
        mu_msq3 = mu_msq.rearrange("p (two b) -> p two b", two=2)
        mu_tc = small.tile([B, CS], DT)         # transpose scratch for the scans
        f_ct = small.tile([CS, B], DT)          # f = var + a*d^2
        f_tc = small.tile([B, CS], DT)
        rb = small.tile([P, 2 * B], DT)         # rb[p, t]=rscale; rb[p, B+t]=nbias
        rb3 = rb.rearrange("p (two b) -> p two b", two=2)
        nc.vector.memset(mu_msq, 0.0)
        nc.vector.memset(f_ct, 0.0)

        out_q = [0]

        def stats(t0, L):
            for t in range(t0, t0 + L):
                if t in ACT_STAT:
                    sq = sqpool.tile([P, F], BF)
                    nc.scalar.activation(
                        out=sq, in_=xbig[:, t, :], func=Act.Identity,
                        accum_out=s2[:, 0, t : t + 1],
                    )
                    sq2 = sqpool.tile([P, F], BF)
                    nc.scalar.activation(
                        out=sq2, in_=xbig[:, t, :], func=Act.Square,
                        accum_out=s2[:, 1, t : t + 1],
                    )
                else:
                    # mean+var per partition-block in ONE DVE pass (bn_stats)
                    nc.vector.bn_stats(
                        out=st6[:, t, 0, :], in_=xbig[:, t, 0 : F // 2]
                    )
                    nc.vector.bn_stats(
                        out=st6[:, t, 1, :], in_=xbig[:, t, F // 2 : F]
                    )


        def scan_and_norm(t0, L):
            cols = slice(t0, t0 + L)
            vcols = slice(B + t0, B + t0 + L)

            ps_stats = psum.tile([CS, 2, LMAX, 2, 2], DT, tag="ps_stats")
            if t0 >= 16:
                # tail: combine q-blocks AND bn chunks straight from raw
                # bn_stats fields (means at [...,1,4], cnt*var at [...,2,5]):
                # mu  = (1/16)  sum mean_field       (4q x 2chunk x 2eo)
                # msq = (1/4096) sum cv + (1/16) sum mean^2
                st6r = st6.rearrange("p b c (k three) -> p b c k three", three=3)
                means = st6r[:, cols, :, :, 1]      # [P, L, 2, 2]
                cvs = st6r[:, cols, :, :, 2]
                m2c = gpool.tile([P, LMAX, 2, 2], BF, tag="m2c")
                nc.vector.tensor_mul(out=m2c[:, 0:L, :, :], in0=means, in1=means)
                nc.tensor.matmul(
                    out=ps_stats[:, 0, 0:L, :, :], lhsT=sb_mask16, rhs=means,
                    start=True, stop=True,
                )
                nc.tensor.matmul(
                    out=ps_stats[:, 1, 0:L, :, :], lhsT=sb_mask4k, rhs=cvs,
                    start=True, stop=False,
                )
                nc.tensor.matmul(
                    out=ps_stats[:, 1, 0:L, :, :], lhsT=sb_mask16,
                    rhs=m2c[:, 0:L, :, :], start=False, stop=True,
                )
                ps_c4 = ps_stats.rearrange("p two l a b -> p two l (a b)")
                nc.vector.tensor_reduce(
                    out=mu_msq3[:, :, cols], in_=ps_c4[:, :, 0:L, :],
                    axis=Ax.X, op=Alu.add,
                )
            else:
                # early groups: combine raw bn_stats fields without touching
                # the DVE spine.  The 4 per-sample (chunk x even/odd) field
                # values sit at uniform stride 3; GPSIMD pre-sums them, PE
                # combines q-blocks with 3 accumulated matmuls.
                st6v = st6.rearrange("p b c (k three) -> p b (c k) three", three=3)
                ms = gpool.tile([P, 4, LMAX], BF, tag="ms")
                # ms[:,0]=sum means; ms[:,1]=sum cv; ms[:,2]=sum mean^2
                nc.gpsimd.tensor_add(
                    out=ms[:, 3, 0:L], in0=st6v[:, cols, 0, 1],
                    in1=st6v[:, cols, 1, 1],
                )
                nc.gpsimd.tensor_add(
                    out=ms[:, 0, 0:L], in0=st6v[:, cols, 2, 1],
                    in1=st6v[:, cols, 3, 1],
                )
                nc.gpsimd.tensor_add(
                    out=ms[:, 0, 0:L], in0=ms[:, 0, 0:L], in1=ms[:, 3, 0:L]
                )
                nc.gpsimd.tensor_add(
                    out=ms[:, 3, 0:L], in0=st6v[:, cols, 0, 2],
                    in1=st6v[:, cols, 1, 2],
                )
                nc.gpsimd.tensor_add(
                    out=ms[:, 1, 0:L], in0=st6v[:, cols, 2, 2],
                    in1=st6v[:, cols, 3, 2],
                )
                nc.gpsimd.tensor_add(
                    out=ms[:, 1, 0:L], in0=ms[:, 1, 0:L], in1=ms[:, 3, 0:L]
                )
                m2q = gpool.tile([P, 4, LMAX], BF, tag="m2q")
                for j in range(4):
                    nc.gpsimd.tensor_mul(
                        out=m2q[:, j, 0:L], in0=st6v[:, cols, j, 1],
                        in1=st6v[:, cols, j, 1],
                    )
                nc.gpsimd.tensor_add(
                    out=m2q[:, 0, 0:L], in0=m2q[:, 0, 0:L], in1=m2q[:, 1, 0:L]
                )
                nc.gpsimd.tensor_add(
                    out=m2q[:, 2, 0:L], in0=m2q[:, 2, 0:L], in1=m2q[:, 3, 0:L]
                )
                nc.gpsimd.tensor_add(
                    out=ms[:, 2, 0:L], in0=m2q[:, 0, 0:L], in1=m2q[:, 2, 0:L]
                )
                nc.tensor.matmul(
                    out=ps_stats[:, 0, 0:L, 0, 0], lhsT=sb_mask16,
                    rhs=ms[:, 0, 0:L], start=True, stop=True,
                )
                nc.tensor.matmul(
                    out=ps_stats[:, 1, 0:L, 0, 0], lhsT=sb_mask4k,
                    rhs=ms[:, 1, 0:L], start=True, stop=False,
                )
                nc.tensor.matmul(
                    out=ps_stats[:, 1, 0:L, 0, 0], lhsT=sb_mask16,
                    rhs=ms[:, 2, 0:L], start=False, stop=True,
                )
                nc.scalar.copy(
                    out=mu_msq3[:, :, cols], in_=ps_stats[:, :, 0:L, 0, 0]
                )

            # ---- s_mu_{t-1} for this group's t-range, directly in ct ----
            # out[c,t] = sum_i mu_tc[i,c] * w[i,t]; cols beyond the prefix are
            # zeros and W kills rows >= t anyway
            ps_mt = psum.tile([B, CS], DT, tag="ps_t")
            nc.tensor.transpose(out=ps_mt, in_=mu_msq[:, 0:B], identity=sb_eye)
            nc.scalar.copy(out=mu_tc, in_=ps_mt)
            ps_smu = psum.tile([CS, LMAX], DT, tag="ps_sv")
            nc.tensor.matmul(
                out=ps_smu[:, 0:L], lhsT=mu_tc, rhs=sb_w[:, cols],
                start=True, stop=False,
            )
            nc.tensor.matmul(
                out=ps_smu[:, 0:L], lhsT=sb_eye, rhs=sb_init[:, cols],
                start=False, stop=True,
            )
            smu_g = gpool.tile([CS, LMAX], DT, tag="smu_g")
            nc.scalar.copy(out=smu_g[:, 0:L], in_=ps_smu[:, 0:L])

            # ---- f = (msq - mu^2) + a*(mu - smu)^2  (all [CS, L], ct) ----
            ve = nc.vector if t0 >= 16 else nc.gpsimd
            mu_cols = mu_msq[:, cols]
            m2 = gpool.tile([CS, LMAX], DT, tag="m2")
            ve.tensor_mul(out=m2[:, 0:L], in0=mu_cols, in1=mu_cols)
            var_g = gpool.tile([CS, LMAX], DT, tag="var_g")
            ve.tensor_sub(
                out=var_g[:, 0:L], in0=mu_msq[:, vcols], in1=m2[:, 0:L]
            )
            d_g = gpool.tile([CS, LMAX], DT, tag="d_g")
            ve.tensor_sub(out=d_g[:, 0:L], in0=mu_cols, in1=smu_g[:, 0:L])
            d2_g = gpool.tile([CS, LMAX], DT, tag="d2_g")
            ve.tensor_mul(out=d2_g[:, 0:L], in0=d_g[:, 0:L], in1=d_g[:, 0:L])
            if ve is nc.vector:
                ve.scalar_tensor_tensor(
                    out=f_ct[:, cols],
                    in0=d2_g[:, 0:L],
                    scalar=AFWD,
                    in1=var_g[:, 0:L],
                    op0=Alu.mult,
                    op1=Alu.add,
                )
            else:
                # gpsimd has no scalar_tensor_tensor: fold a into d2, then add
                ve.tensor_scalar(
                    out=d2_g[:, 0:L], in0=d2_g[:, 0:L],
                    scalar1=AFWD, scalar2=None, op0=Alu.mult,
                )
                ve.tensor_add(
                    out=f_ct[:, cols], in0=d2_g[:, 0:L], in1=var_g[:, 0:L]
                )

            # ---- s_var_{t-1} via the same W contraction on f, direct ct ----
            ps_ft = psum.tile([B, CS], DT, tag="ps_t")
            nc.tensor.transpose(out=ps_ft, in_=f_ct, identity=sb_eye)
            nc.scalar.copy(out=f_tc, in_=ps_ft)
            ps_svar = psum.tile([CS, LMAX], DT, tag="ps_sv")
            nc.tensor.matmul(
                out=ps_svar[:, 0:L], lhsT=f_tc, rhs=sb_w[:, cols],
                start=True, stop=False,
            )
            nc.tensor.matmul(
                out=ps_svar[:, 0:L], lhsT=sb_eye, rhs=sb_init[:, vcols],
                start=False, stop=True,
            )

            # ---- rscale = 1/sqrt(svar+eps) in ONE ACT op, from PSUM ----
            rs_g = gpool.tile([CS, LMAX], DT, tag="rs_g")
            nc.scalar.activation(
                out=rs_g[:, 0:L],
                in_=ps_svar[:, 0:L],
                func=Act.Abs_reciprocal_sqrt,
                bias=sb_eps,
                scale=1.0,
            )
            nb_g = gpool.tile([CS, LMAX], DT, tag="nb_g")
            if ve is nc.vector:
                ve.scalar_tensor_tensor(
                    out=nb_g[:, 0:L],
                    in0=smu_g[:, 0:L],
                    scalar=-1.0,
                    in1=rs_g[:, 0:L],
                    op0=Alu.mult,
                    op1=Alu.mult,
                )
            else:
                ve.tensor_scalar(
                    out=nb_g[:, 0:L], in0=smu_g[:, 0:L],
                    scalar1=-1.0, scalar2=None, op0=Alu.mult,
                )
                ve.tensor_mul(
                    out=nb_g[:, 0:L], in0=nb_g[:, 0:L], in1=rs_g[:, 0:L]
                )

            # ---- broadcast to all 128 partitions via PE ----
            ps_rb = psum.tile([P, 2, LMAX], DT, tag="ps_rb")
            nc.tensor.matmul(
                out=ps_rb[:, 0, 0:L],
                lhsT=sb_bmask,
                rhs=rs_g[:, 0:L],
                start=True,
                stop=True,
            )
            nc.tensor.matmul(
                out=ps_rb[:, 1, 0:L],
                lhsT=sb_bmask,
                rhs=nb_g[:, 0:L],
                start=True,
                stop=True,
            )
            nc.scalar.copy(out=rb3[:, :, cols], in_=ps_rb[:, :, 0:L])

            # ---- normalize in place + stream out per pair ----
            for t in range(t0, t0 + L):
                if t in NORM_DVE:
                    continue      # deferred to the DVE epilogue after the spine
                if t in NORM_GPS:
                    nc.gpsimd.tensor_scalar(
                        out=xbig[:, t, :],
                        in0=xbig[:, t, :],
                        scalar1=rb[:, t : t + 1],
                        scalar2=rb[:, B + t : B + t + 1],
                        op0=Alu.mult,
                        op1=Alu.add,
                    )
                else:
                    nc.scalar.activation(
                        out=xbig[:, t, :],
                        in_=xbig[:, t, :],
                        func=Act.Identity,
                        bias=rb[:, B + t : B + t + 1],
                        scale=rb[:, t : t + 1],
                    )
                if t % 2 == 1:
                    pr = slice(t - 1, t + 1)
                    # alternate SWDGE (gpsimd) / HWDGE (sync) trigger queues
                    eng = nc.gpsimd if out_q[0] % 2 == 0 else nc.sync
                    eng.dma_start(out=out_h[:, pr, :], in_=xbig[:, pr, :])
                    out_q[0] += 1

        # software-pipelined: stats run two groups ahead of scan+norm
        spans = []
        t0 = 0
        for L in GROUPS:
            spans.append((t0, L))
            t0 += L
        DEPTH = 2
        for i, (s0, sl) in enumerate(spans):
            stats(s0, sl)
            if i >= DEPTH:
                scan_and_norm(*spans[i - DEPTH])
        for i in range(len(spans) - DEPTH, len(spans)):
            scan_and_norm(*spans[i])

        # ---- DVE norm epilogue: rb for these groups is long ready, so the
        # bn spine never stalls and these stream back-to-back at 2x rate ----
        for t in sorted(NORM_DVE):
            nc.vector.tensor_scalar(
                out=xbig[:, t, :],
                in0=xbig[:, t, :],
                scalar1=rb[:, t : t + 1],
                scalar2=rb[:, B + t : B + t + 1],
                op0=Alu.mult,
                op1=Alu.add,
            )
            if t % 2 == 1:
                pr = slice(t - 1, t + 1)
                # sync only: a GPSIMD drain at teardown can stall its queue
                nc.sync.dma_start(out=out_h[:, pr, :], in_=xbig[:, pr, :])

    nc.compile()
    return nc


def _blob():
    i = np.arange(B)[:, None].astype(np.float64)
    t = np.arange(B)[None, :].astype(np.float64)
    w = np.where(i < t, (1.0 - AFWD) * AFWD ** (t - 1.0 - i), 0.0).astype(np.float32)
    mask = np.zeros((P, CS), np.float32)
    mask[np.arange(P), np.arange(P) % CS] = 1.0 / Q
    bmask = np.zeros((CS, P), np.float32)
    bmask[np.arange(P) % CS, np.arange(P)] = 1.0
    blob = np.zeros((P, CBLOB), np.float32)
    blob[0:B, OFF_W : OFF_W + B] = w
    blob[:, OFF_MASK : OFF_MASK + CS] = mask
    blob[0:CS, OFF_BMASK : OFF_BMASK + P] = bmask
    blob[0:CS, OFF_EYE : OFF_EYE + CS] = np.eye(CS, dtype=np.float32)
    return blob


def kernel(**inputs):
    global LAST_EXEC_NS, LAST_RESULTS
    import ml_dtypes

    x = np.asarray(inputs["x"], dtype=np.float32)
    mu0 = np.asarray(inputs["mu0"], dtype=np.float32)
    var0 = np.asarray(inputs["var0"], dtype=np.float32)
    assert x.shape == (B, H, W_SP, C)

    from concourse.bass_utils import run_bass_kernel_spmd

    if "nc" not in _COMPILED:
        _COMPILED["nc"] = _build_bass()
    nc = _COMPILED["nc"]

    apow = (AFWD ** np.arange(B, dtype=np.float64)).astype(np.float32)[None, :]

    # [B, Q, F, C] view of x; per-core shard is [Q, CS, B, F] -> [P, B, F]
    x16 = x.astype(ml_dtypes.bfloat16)
    xr = x16.reshape(B, Q, F, C)
    in_maps = []
    for core in range(NCORES):
        c0 = core * CS
        xs = np.ascontiguousarray(
            xr[:, :, :, c0 : c0 + CS].transpose(1, 3, 0, 2)
        ).reshape(P, B, F)
        blob = _blob()
        blob[0:CS, OFF_INIT : OFF_INIT + B] = mu0[c0 : c0 + CS, None] * apow
        blob[0:CS, OFF_INIT + B : OFF_INIT + 2 * B] = (
            var0[c0 : c0 + CS, None] * apow
        )
        import ml_dtypes as _md
        maskb = np.zeros((P, 3 * CS), np.float32)
        maskb[np.arange(P), np.arange(P) % CS] = 1.0 / Q
        maskb[np.arange(P), CS + np.arange(P) % CS] = 1.0 / 16.0
        maskb[np.arange(P), 2 * CS + np.arange(P) % CS] = 1.0 / 4096.0
        in_maps.append(
            {"x": xs, "blob": blob, "maskb": maskb.astype(_md.bfloat16)}
        )

    trace = bool(int(os.environ.get("NORM_KERNEL_TRACE", "0")))
    if trace:
        _ensure_ntff_hook()
    res = run_bass_kernel_spmd(nc, in_maps, list(range(NCORES)), trace=trace)
    LAST_EXEC_NS = res.exec_time_ns
    LAST_RESULTS = res

    out = np.empty((B, Q, F, C), np.float32)
    for core in range(NCORES):
        c0 = core * CS
        o = np.asarray(res.results[core]["out"]).astype(np.float32)
        o = o.reshape(Q, CS, B, F)
        out[:, :, :, c0 : c0 + CS] = o.transpose(2, 0, 3, 1)
    return out.reshape(B, H, W_SP, C)


# revision 54
# speedup vs baseline: 1.0337x; 1.0337x over previous
"""Online Normalization (forward) on 8 Trainium2 NeuronCores.

Reference semantics (per batch sample t, stats per channel over H*W):
    out_t = (x_t - s_mu_{t-1}) / sqrt(s_var_{t-1} + eps)
    mu_t  = mean(x_t);  var_t = mean(x_t^2) - mu_t^2
    s_mu_t  = a*s_mu_{t-1}  + (1-a)*mu_t
    s_var_t = a*s_var_{t-1} + (1-a)*var_t + a*(1-a)*(mu_t - s_mu_{t-1})^2

The EMA recurrence is linear, so instead of a sequential scan over the batch
axis we compute per-sample batch stats in parallel and apply the recurrence
as small matmuls on the tensor engine:
    s_mu_{t-1}  = a^t mu0  + sum_i W[i,t] * mu_i,   W[i,t] = (1-a) a^{t-1-i}, i<t
    s_var_{t-1} = a^t var0 + sum_i W[i,t] * f_i,    f_i = var_i + a*d_i^2,
                                                    d_i = mu_i - s_mu_{i-1}

v22: x and out are staged in bf16 (tolerance is 2e-2; bf16 staging costs
~2e-3), halving DMA traffic; the const blob loads FIRST so the scan can
start as soon as the first samples land.  The DVE runs a pure bn_stats
"spine" (mean+var per sample in one pass) with NOTHING else queued on it,
so it streams gap-free; the scan chain runs entirely off-spine (PSUM->SBUF
copies and 1/sqrt on ACT, init terms accumulated on the PE, f-recurrence
small ops on GPSIMD, transposes on the PE).  Early samples normalize on
ACT/GPSIMD while the spine runs; the whole tail (t>=16) is ONE fused scan
group whose normalizes stream on DVE right after the spine drains, with
their output DMAs on the SP queue (a GPSIMD teardown drain would stall the
SWDGE queue).  The tail group also skips bn_aggr entirely -- its combine
matmuls consume the raw bn_stats fields (means/cnt*var) with 1/16 and
1/4096 masks accumulated in PSUM -- and its recurrence small-ops run on the
now-idle DVE instead of GPSIMD, shortening the one off-spine chain.

Sharding: channels C=256 split across 8 cores (32 each) -- every channel's
recurrence is independent. Per core the shard sits resident in SBUF as
[128 partitions, 32 t, 1024 f] bf16, partition p = q*32 + c (q = one of 4
spatial blocks, c = channel).
"""

import os
import sys

import numpy as np

sys.path.insert(0, "/opt/trn_rl_repo")

B = 32          # batch (sequential scan axis)
H = 64
W_SP = 64
C = 256
NCORES = 8
CS = C // NCORES    # 32 channels per core
Q = 4               # spatial blocks per sample
F = (H * W_SP) // Q  # 1024 elements per block
P = 128             # partitions (Q*CS)
AFWD = 0.999
EPS = 1e-5
# scan groups (samples per scan matmul batch); tapered head for early output
GROUPS = [2, 6, 8, 16]
assert sum(GROUPS) == B

# engine assignment by sample index.  ACT_STAT samples get their stats from
# two ACT accumulate passes (Identity -> sum, Square -> sumsq) instead of DVE
# bn_stats, offloading the DVE stat stream; they sit at the tails of groups
# 1-3 so the ACT work overlaps DVE's bn_stats of the same group's head.
ACT_STAT = set()                            # pure bn_stats spine on DVE
NORM_GPS = {3, 7, 11, 15}                   # one GPSIMD norm per odd pair
NORM_DVE = set(range(16, 32))               # tail norms: DVE epilogue after spine

# const blob free-axis layout (f32, [128, CBLOB])
OFF_W = 0            # w     [B, B]     rows 0..31
OFF_MASK = OFF_W + B      # mask  [P, CS]
OFF_BMASK = OFF_MASK + CS  # bmask [CS, P]  rows 0..31
OFF_INIT = OFF_BMASK + P  # init  [CS, 2B] rows 0..31
OFF_EYE = OFF_INIT + 2 * B  # eye   [CS, CS] rows 0..31
CBLOB = OFF_EYE + CS

LAST_EXEC_NS = None
LAST_RESULTS = None
_COMPILED = {}


def _ensure_ntff_hook():
    """The axon boot degrades silently when ``antenv.axon_hooks`` is missing;
    provide the module + the ctypes-based NRT-profile hook ourselves so
    ``run_bass_kernel_spmd(trace=True)`` can capture NTFF profiles."""
    try:
        from antenv.axon_hooks import get_axon_ntff_profile_hook  # noqa: F401

        return
    except ImportError:
        pass

    import contextlib
    import ctypes
    import types

    so_path = "/opt/axon/libaxon_pjrt.so"
    state = {"hook": None}

    mod = types.ModuleType("antenv.axon_hooks")

    def set_axon_ntff_profile_hook(h):
        state["hook"] = h

    def get_axon_ntff_profile_hook():
        return state["hook"]

    mod.set_axon_ntff_profile_hook = set_axon_ntff_profile_hook
    mod.get_axon_ntff_profile_hook = get_axon_ntff_profile_hook
    import antenv

    antenv.axon_hooks = mod
    sys.modules["antenv.axon_hooks"] = mod

    if not os.path.exists(so_path):
        return
    lib = ctypes.CDLL(so_path)
    if not hasattr(lib, "axon_start_nrt_profile"):
        return
    lib.axon_start_nrt_profile.argtypes = [
        ctypes.POINTER(ctypes.c_int64),
        ctypes.c_size_t,
    ]
    lib.axon_start_nrt_profile.restype = ctypes.c_int64
    lib.axon_stop_nrt_profile.argtypes = [ctypes.c_char_p]
    lib.axon_stop_nrt_profile.restype = ctypes.c_int64

    @contextlib.contextmanager
    def _hook(output_dir, device_ids):
        import jax

        jax.devices()
        if device_ids:
            ids = (ctypes.c_int64 * len(device_ids))(*device_ids)
            rc = lib.axon_start_nrt_profile(ids, len(device_ids))
        else:
            rc = lib.axon_start_nrt_profile(None, 0)
        if rc != 0:
            raise RuntimeError(f"axon_start_nrt_profile rc={rc}")
        try:
            yield
        finally:
            n = lib.axon_stop_nrt_profile(str(output_dir).encode())
            print(f"profile: {n} file(s) written to {output_dir}", file=sys.stderr)

    state["hook"] = _hook


def _build_bass():
    from contextlib import ExitStack

    import concourse.bacc as bacc
    import concourse.tile as tile
    from concourse import mybir

    DT = mybir.dt.float32
    BF = mybir.dt.bfloat16
    Alu = mybir.AluOpType
    Act = mybir.ActivationFunctionType
    Ax = mybir.AxisListType

    nc = bacc.Bacc(
        "TRN2", target_bir_lowering=False, debug=False, num_devices=NCORES
    )
    x_h = nc.declare_dram_parameter("x", [P, B, F], BF, isOutput=False)
    blob_h = nc.declare_dram_parameter("blob", [P, CBLOB], DT, isOutput=False)
    maskb_h = nc.declare_dram_parameter("maskb", [P, 3 * CS], BF, isOutput=False)
    out_h = nc.declare_dram_parameter("out", [P, B, F], BF, isOutput=True)

    LMAX = max(GROUPS)
    NPAIR = B // 2

    with tile.TileContext(nc) as tc, ExitStack() as ctx:
        consts = ctx.enter_context(tc.tile_pool(name="consts", bufs=1))
        xpool = ctx.enter_context(tc.tile_pool(name="xp", bufs=1))
        sqpool = ctx.enter_context(tc.tile_pool(name="sqp", bufs=3))
        small = ctx.enter_context(tc.tile_pool(name="small", bufs=1))
        gpool = ctx.enter_context(tc.tile_pool(name="gp", bufs=3))
        psum = ctx.enter_context(tc.tile_pool(name="ps", bufs=2, space="PSUM"))

        blob = consts.tile([P, CBLOB], DT)
        sb_maskb3 = consts.tile([P, 3 * CS], BF)  # /Q | /16 | /4096 in bf16
        sb_maskb = sb_maskb3[:, 0:CS]
        sb_mask16 = sb_maskb3[:, CS : 2 * CS]
        sb_mask4k = sb_maskb3[:, 2 * CS : 3 * CS]
        sb_w = blob[0:B, OFF_W : OFF_W + B]          # [B, B]
        sb_mask = blob[:, OFF_MASK : OFF_MASK + CS]  # [P, CS]  (p%CS==c)/(Q*F)
        sb_bmask = blob[0:CS, OFF_BMASK : OFF_BMASK + P]  # [CS, P]
        sb_init = blob[0:CS, OFF_INIT : OFF_INIT + 2 * B]  # [CS, 2B]
        sb_eye = blob[0:CS, OFF_EYE : OFF_EYE + CS]  # [CS, CS] identity

        xbig = xpool.tile([P, B, F], BF)        # resident shard, 64 KiB/partition
        # ---- all input DMAs up front (SP queue); sample 0 triggers before
        # the const blobs (consts aren't needed until the first scan ~+6us)
        nc.sync.dma_start(out=xbig[:, 0, 0 : F // 2], in_=x_h[:, 0, 0 : F // 2])
        nc.sync.dma_start(out=xbig[:, 0, F // 2 : F], in_=x_h[:, 0, F // 2 : F])
        nc.sync.dma_start(out=blob, in_=blob_h[:, :])
        nc.sync.dma_start(out=sb_maskb3, in_=maskb_h[:, :])
        nc.sync.dma_start(out=xbig[:, 1:2, :], in_=x_h[:, 1:2, :])
        # singles for t2..5 -- finer landing granularity while the DMA
        # engines ramp, so the bn spine waits less for early samples
        for t in range(2, 6):
            nc.sync.dma_start(out=xbig[:, t : t + 1, :], in_=x_h[:, t : t + 1, :])
        for k in range(3, NPAIR):
            nc.sync.dma_start(
                out=xbig[:, 2 * k : 2 * k + 2, :], in_=x_h[:, 2 * k : 2 * k + 2, :]
            )

        sb_eps = consts.tile([CS, 1], DT)
        nc.vector.memset(sb_eps, EPS)

        st6 = small.tile([P, B, 2, 6], BF)      # bn_stats out, 2 chunks/sample
        mv = small.tile([P, 2, B], BF)          # [,0,t]=mean  [,1,t]=var (pre-fix)
        s2 = small.tile([P, 2, B], DT)          # ACT_STAT raw sums / sumsq
        # Scan state in ct layout ([channel, t]): per-group writes slice the
        # FREE axis (partition slices must start at 0 on compute engines).
        mu_msq = small.tile([CS, 2 * B], DT)    # cols t: mu_ct; cols B+t: msq_ct
        mu_msq3 = mu_msq.rearrange("p (two b) -> p two b", two=2)
        mu_tc = small.tile([B, CS], DT)         # transpose scratch for the scans
        f_ct = small.tile([CS, B], DT)          # f = var + a*d^2
        f_tc = small.tile([B, CS], DT)
        rb = small.tile([P, 2 * B], DT)         # rb[p, t]=rscale; rb[p, B+t]=nbias
        rb3 = rb.rearrange("p (two b) -> p two b", two=2)
        nc.vector.memset(mu_msq, 0.0)
        nc.vector.memset(f_ct, 0.0)

        out_q = [0]

        def stats(t0, L):
            for t in range(t0, t0 + L):
                if t in ACT_STAT:
                    sq = sqpool.tile([P, F], BF)
                    nc.scalar.activation(
                        out=sq, in_=xbig[:, t, :], func=Act.Identity,
                        accum_out=s2[:, 0, t : t + 1],
                    )
                    sq2 = sqpool.tile([P, F], BF)
                    nc.scalar.activation(
                        out=sq2, in_=xbig[:, t, :], func=Act.Square,
                        accum_out=s2[:, 1, t : t + 1],
                    )
                else:
                    # mean+var per partition-block in ONE DVE pass (bn_stats)
                    nc.vector.bn_stats(
                        out=st6[:, t, 0, :], in_=xbig[:, t, 0 : F // 2]
                    )
                    nc.vector.bn_stats(
                        out=st6[:, t, 1, :], in_=xbig[:, t, F // 2 : F]
                    )


        def scan_and_norm(t0, L):
            cols = slice(t0, t0 + L)
            vcols = slice(B + t0, B + t0 + L)

            ps_stats = psum.tile([CS, 2, LMAX, 2, 2], DT, tag="ps_stats")
            if t0 >= 16:
                # tail: combine q-blocks AND bn chunks straight from raw
                # bn_stats fields (means at [...,1,4], cnt*var at [...,2,5]):
                # mu  = (1/16)  sum mean_field       (4q x 2chunk x 2eo)
                # msq = (1/4096) sum cv + (1/16) sum mean^2
                st6r = st6.rearrange("p b c (k three) -> p b c k three", three=3)
                means = st6r[:, cols, :, :, 1]      # [P, L, 2, 2]
                cvs = st6r[:, cols, :, :, 2]
                m2c = gpool.tile([P, LMAX, 2, 2], BF, tag="m2c")
                nc.vector.tensor_mul(out=m2c[:, 0:L, :, :], in0=means, in1=means)
                nc.tensor.matmul(
                    out=ps_stats[:, 0, 0:L, :, :], lhsT=sb_mask16, rhs=means,
                    start=True, stop=True,
                )
                nc.tensor.matmul(
                    out=ps_stats[:, 1, 0:L, :, :], lhsT=sb_mask4k, rhs=cvs,
                    start=True, stop=False,
                )
                nc.tensor.matmul(
                    out=ps_stats[:, 1, 0:L, :, :], lhsT=sb_mask16,
                    rhs=m2c[:, 0:L, :, :], start=False, stop=True,
                )
                ps_c4 = ps_stats.rearrange("p two l a b -> p two l (a b)")
                nc.vector.tensor_reduce(
                    out=mu_msq3[:, :, cols], in_=ps_c4[:, :, 0:L, :],
                    axis=Ax.X, op=Alu.add,
                )
            else:
                # early groups: combine raw bn_stats fields without touching
                # the DVE spine.  The 4 per-sample (chunk x even/odd) field
                # values sit at uniform stride 3; GPSIMD pre-sums them, PE
                # combines q-blocks with 3 accumulated matmuls.
                st6v = st6.rearrange("p b c (k three) -> p b (c k) three", three=3)
                ms = gpool.tile([P, 4, LMAX], BF, tag="ms")
                # ms[:,0]=sum means; ms[:,1]=sum cv; ms[:,2]=sum mean^2
                nc.gpsimd.tensor_add(
                    out=ms[:, 3, 0:L], in0=st6v[:, cols, 0, 1],
                    in1=st6v[:, cols, 1, 1],
                )
                nc.gpsimd.tensor_add(
                    out=ms[:, 0, 0:L], in0=st6v[:, cols, 2, 1],
                    in1=st6v[:, cols, 3, 1],
                )
                nc.gpsimd.tensor_add(
                    out=ms[:, 0, 0:L], in0=ms[:, 0, 0:L], in1=ms[:, 3, 0:L]
                )
                nc.gpsimd.tensor_add(
                    out=ms[:, 3, 0:L], in0=st6v[:, cols, 0, 2],
                    in1=st6v[:, cols, 1, 2],
                )
                nc.gpsimd.tensor_add(
                    out=ms[:, 1, 0:L], in0=st6v[:, cols, 2, 2],
                    in1=st6v[:, cols, 3, 2],
                )
                nc.gpsimd.tensor_add(
                    out=ms[:, 1, 0:L], in0=ms[:, 1, 0:L], in1=ms[:, 3, 0:L]
                )
                m2q = gpool.tile([P, 4, LMAX], BF, tag="m2q")
                for j in range(4):
                    nc.gpsimd.tensor_mul(
                        out=m2q[:, j, 0:L], in0=st6v[:, cols, j, 1],
                        in1=st6v[:, cols, j, 1],
                    )
                nc.gpsimd.tensor_add(
                    out=m2q[:, 0, 0:L], in0=m2q[:, 0, 0:L], in1=m2q[:, 1, 0:L]
                )
                nc.gpsimd.tensor_add(
                    out=m2q[:, 2, 0:L], in0=m2q[:, 2, 0:L], in1=m2q[:, 3, 0:L]
                )
                nc.gpsimd.tensor_add(
                    out=ms[:, 2, 0:L], in0=m2q[:, 0, 0:L], in1=m2q[:, 2, 0:L]
                )
                nc.tensor.matmul(
                    out=ps_stats[:, 0, 0:L, 0, 0], lhsT=sb_mask16,
                    rhs=ms[:, 0, 0:L], start=True, stop=True,
                )
                nc.tensor.matmul(
                    out=ps_stats[:, 1, 0:L, 0, 0], lhsT=sb_mask4k,
                    rhs=ms[:, 1, 0:L], start=True, stop=False,
                )
                nc.tensor.matmul(
                    out=ps_stats[:, 1, 0:L, 0, 0], lhsT=sb_mask16,
                    rhs=ms[:, 2, 0:L], start=False, stop=True,
                )
                nc.scalar.copy(
                    out=mu_msq3[:, :, cols], in_=ps_stats[:, :, 0:L, 0, 0]
                )

            # ---- s_mu_{t-1} for this group's t-range, directly in ct ----
            # out[c,t] = sum_i mu_tc[i,c] * w[i,t]; cols beyond the prefix are
            # zeros and W kills rows >= t anyway
            ps_mt = psum.tile([B, CS], DT, tag="ps_t")
            nc.tensor.transpose(out=ps_mt, in_=mu_msq[:, 0:B], identity=sb_eye)
            nc.scalar.copy(out=mu_tc, in_=ps_mt)
            ps_smu = psum.tile([CS, LMAX], DT, tag="ps_sv")
            nc.tensor.matmul(
                out=ps_smu[:, 0:L], lhsT=mu_tc, rhs=sb_w[:, cols],
                start=True, stop=False,
            )
            nc.tensor.matmul(
                out=ps_smu[:, 0:L], lhsT=sb_eye, rhs=sb_init[:, cols],
                start=False, stop=True,
            )
            smu_g = gpool.tile([CS, LMAX], DT, tag="smu_g")
            nc.scalar.copy(out=smu_g[:, 0:L], in_=ps_smu[:, 0:L])

            # ---- f = (msq - mu^2) + a*(mu - smu)^2  (all [CS, L], ct) ----
            ve = nc.vector if t0 >= 16 else nc.gpsimd
            mu_cols = mu_msq[:, cols]
            m2 = gpool.tile([CS, LMAX], DT, tag="m2")
            ve.tensor_mul(out=m2[:, 0:L], in0=mu_cols, in1=mu_cols)
            var_g = gpool.tile([CS, LMAX], DT, tag="var_g")
            ve.tensor_sub(
                out=var_g[:, 0:L], in0=mu_msq[:, vcols], in1=m2[:, 0:L]
            )
            d_g = gpool.tile([CS, LMAX], DT, tag="d_g")
            ve.tensor_sub(out=d_g[:, 0:L], in0=mu_cols, in1=smu_g[:, 0:L])
            d2_g = gpool.tile([CS, LMAX], DT, tag="d2_g")
            ve.tensor_mul(out=d2_g[:, 0:L], in0=d_g[:, 0:L], in1=d_g[:, 0:L])
            if ve is nc.vector:
                ve.scalar_tensor_tensor(
                    out=f_ct[:, cols],
                    in0=d2_g[:, 0:L],
                    scalar=AFWD,
                    in1=var_g[:, 0:L],
                    op0=Alu.mult,
                    op1=Alu.add,
                )
            else:
                # gpsimd has no scalar_tensor_tensor: fold a into d2, then add
                ve.tensor_scalar(
                    out=d2_g[:, 0:L], in0=d2_g[:, 0:L],
                    scalar1=AFWD, scalar2=None, op0=Alu.mult,
                )
                ve.tensor_add(
                    out=f_ct[:, cols], in0=d2_g[:, 0:L], in1=var_g[:, 0:L]
                )

            # ---- s_var_{t-1} via the same W contraction on f, direct ct ----
            ps_ft = psum.tile([B, CS], DT, tag="ps_t")
            nc.tensor.transpose(out=ps_ft, in_=f_ct, identity=sb_eye)
            nc.scalar.copy(out=f_tc, in_=ps_ft)
            ps_svar = psum.tile([CS, LMAX], DT, tag="ps_sv")
            nc.tensor.matmul(
                out=ps_svar[:, 0:L], lhsT=f_tc, rhs=sb_w[:, cols],
                start=True, stop=False,
            )
            nc.tensor.matmul(
                out=ps_svar[:, 0:L], lhsT=sb_eye, rhs=sb_init[:, vcols],
                start=False, stop=True,
            )

            # ---- rscale = 1/sqrt(svar+eps) in ONE ACT op, from PSUM ----
            rs_g = gpool.tile([CS, LMAX], DT, tag="rs_g")
            nc.scalar.activation(
                out=rs_g[:, 0:L],
                in_=ps_svar[:, 0:L],
                func=Act.Abs_reciprocal_sqrt,
                bias=sb_eps,
                scale=1.0,
            )
            nb_g = gpool.tile([CS, LMAX], DT, tag="nb_g")
            if ve is nc.vector:
                ve.scalar_tensor_tensor(
                    out=nb_g[:, 0:L],
                    in0=smu_g[:, 0:L],
                    scalar=-1.0,
                    in1=rs_g[:, 0:L],
                    op0=Alu.mult,
                    op1=Alu.mult,
                )
            else:
                ve.tensor_scalar(
                    out=nb_g[:, 0:L], in0=smu_g[:, 0:L],
                    scalar1=-1.0, scalar2=None, op0=Alu.mult,
                )
                ve.tensor_mul(
                    out=nb_g[:, 0:L], in0=nb_g[:, 0:L], in1=rs_g[:, 0:L]
                )

            # ---- broadcast to all 128 partitions via PE ----
            ps_rb = psum.tile([P, 2, LMAX], DT, tag="ps_rb")
            nc.tensor.matmul(
                out=ps_rb[:, 0, 0:L],
                lhsT=sb_bmask,
                rhs=rs_g[:, 0:L],
                start=True,
                stop=True,
            )
            nc.tensor.matmul(
                out=ps_rb[:, 1, 0:L],
                lhsT=sb_bmask,
                rhs=nb_g[:, 0:L],
                start=True,
                stop=True,
            )
            nc.scalar.copy(out=rb3[:, :, cols], in_=ps_rb[:, :, 0:L])

            # ---- normalize in place + stream out per pair ----
            for t in range(t0, t0 + L):
                if t in NORM_DVE:
                    continue      # deferred to the DVE epilogue after the spine
                if t in NORM_GPS:
                    nc.gpsimd.tensor_scalar(
                        out=xbig[:, t, :],
                        in0=xbig[:, t, :],
                        scalar1=rb[:, t : t + 1],
                        scalar2=rb[:, B + t : B + t + 1],
                        op0=Alu.mult,
                        op1=Alu.add,
                    )
                else:
                    nc.scalar.activation(
                        out=xbig[:, t, :],
                        in_=xbig[:, t, :],
                        func=Act.Identity,
                        bias=rb[:, B + t : B + t + 1],
                        scale=rb[:, t : t + 1],
                    )
                if t % 2 == 1:
                    pr = slice(t - 1, t + 1)
                    # alternate SWDGE (gpsimd) / HWDGE (sync) trigger queues
                    eng = nc.gpsimd if out_q[0] % 2 == 0 else nc.sync
                    eng.dma_start(out=out_h[:, pr, :], in_=xbig[:, pr, :])
                    out_q[0] += 1

        # software-pipelined: stats run two groups ahead of scan+norm
        spans = []
        t0 = 0
        for L in GROUPS:
            spans.append((t0, L))
            t0 += L
        DEPTH = 2
        for i, (s0, sl) in enumerate(spans):
            stats(s0, sl)
            if i >= DEPTH:
                scan_and_norm(*spans[i - DEPTH])
        for i in range(len(spans) - DEPTH, len(spans)):
            scan_and_norm(*spans[i])

        # ---- DVE norm epilogue: rb for these groups is long ready, so the
        # bn spine never stalls and these stream back-to-back at 2x rate ----
        for t in sorted(NORM_DVE):
            nc.vector.tensor_scalar(
                out=xbig[:, t, :],
                in0=xbig[:, t, :],
                scalar1=rb[:, t : t + 1],
                scalar2=rb[:, B + t : B + t + 1],
                op0=Alu.mult,
                op1=Alu.add,
            )
            if t % 2 == 1:
                pr = slice(t - 1, t + 1)
                # sync only: a GPSIMD drain at teardown can stall its queue
                nc.sync.dma_start(out=out_h[:, pr, :], in_=xbig[:, pr, :])

    nc.compile()
    return nc


def _blob():
    i = np.arange(B)[:, None].astype(np.float64)
    t = np.arange(B)[None, :].astype(np.float64)
    w = np.where(i < t, (1.0 - AFWD) * AFWD ** (t - 1.0 - i), 0.0).astype(np.float32)
    mask = np.zeros((P, CS), np.float32)
    mask[np.arange(P), np.arange(P) % CS] = 1.0 / Q
    bmask = np.zeros((CS, P), np.float32)
    bmask[np.arange(P) % CS, np.arange(P)] = 1.0
    blob = np.zeros((P, CBLOB), np.float32)
    blob[0:B, OFF_W : OFF_W + B] = w
    blob[:, OFF_MASK : OFF_MASK + CS] = mask
    blob[0:CS, OFF_BMASK : OFF_BMASK + P] = bmask
    blob[0:CS, OFF_EYE : OFF_EYE + CS] = np.eye(CS, dtype=np.float32)
    return blob


def kernel(**inputs):
    global LAST_EXEC_NS, LAST_RESULTS
    import ml_dtypes

    x = np.asarray(inputs["x"], dtype=np.float32)
    mu0 = np.asarray(inputs["mu0"], dtype=np.float32)
    var0 = np.asarray(inputs["var0"], dtype=np.float32)
    assert x.shape == (B, H, W_SP, C)

    from concourse.bass_utils import run_bass_kernel_spmd

    if "nc" not in _COMPILED:
        _COMPILED["nc"] = _build_bass()
    nc = _COMPILED["nc"]

    apow = (AFWD ** np.arange(B, dtype=np.float64)).astype(np.float32)[None, :]

    # [B, Q, F, C] view of x; per-core shard is [Q, CS, B, F] -> [P, B, F]
    x16 = x.astype(ml_dtypes.bfloat16)
    xr = x16.reshape(B, Q, F, C)
    in_maps = []
    for core in range(NCORES):
        c0 = core * CS
        xs = np.ascontiguousarray(
            xr[:, :, :, c0 : c0 + CS].transpose(1, 3, 0, 2)
        ).reshape(P, B, F)
        blob = _blob()
        blob[0:CS, OFF_INIT : OFF_INIT + B] = mu0[c0 : c0 + CS, None] * apow
        blob[0:CS, OFF_INIT + B : OFF_INIT + 2 * B] = (
            var0[c0 : c0 + CS, None] * apow
        )
        import ml_dtypes as _md
        maskb = np.zeros((P, 3 * CS), np.float32)
        maskb[np.arange(P), np.arange(P) % CS] = 1.0 / Q
        maskb[np.arange(P), CS + np.arange(P) % CS] = 1.0 / 16.0
        maskb[np.arange(P), 2 * CS + np.arange(P) % CS] = 1.0 / 4096.0
        in_maps.append(
            {"x": xs, "blob": blob, "maskb": maskb.astype(_md.bfloat16)}
        )

    trace = bool(int(os.environ.get("NORM_KERNEL_TRACE", "0")))
    if trace:
        _ensure_ntff_hook()
    res = run_bass_kernel_spmd(nc, in_maps, list(range(NCORES)), trace=trace)
    LAST_EXEC_NS = res.exec_time_ns
    LAST_RESULTS = res

    out = np.empty((B, Q, F, C), np.float32)
    for core in range(NCORES):
        c0 = core * CS
        o = np.asarray(res.results[core]["out"]).astype(np.float32)
        o = o.reshape(Q, CS, B, F)
        out[:, :, :, c0 : c0 + CS] = o.transpose(2, 0, 3, 1)
    return out.reshape(B, H, W_SP, C)


# revision 55
# speedup vs baseline: 1.0438x; 1.0098x over previous
"""Online Normalization (forward) on 8 Trainium2 NeuronCores.

Reference semantics (per batch sample t, stats per channel over H*W):
    out_t = (x_t - s_mu_{t-1}) / sqrt(s_var_{t-1} + eps)
    mu_t  = mean(x_t);  var_t = mean(x_t^2) - mu_t^2
    s_mu_t  = a*s_mu_{t-1}  + (1-a)*mu_t
    s_var_t = a*s_var_{t-1} + (1-a)*var_t + a*(1-a)*(mu_t - s_mu_{t-1})^2

The EMA recurrence is linear, so instead of a sequential scan over the batch
axis we compute per-sample batch stats in parallel and apply the recurrence
as small matmuls on the tensor engine:
    s_mu_{t-1}  = a^t mu0  + sum_i W[i,t] * mu_i,   W[i,t] = (1-a) a^{t-1-i}, i<t
    s_var_{t-1} = a^t var0 + sum_i W[i,t] * f_i,    f_i = var_i + a*d_i^2,
                                                    d_i = mu_i - s_mu_{i-1}

v22: x and out are staged in bf16 (tolerance is 2e-2; bf16 staging costs
~2e-3), halving DMA traffic; the const blob loads FIRST so the scan can
start as soon as the first samples land.  The DVE runs a pure bn_stats
"spine" (mean+var per sample in one pass) with NOTHING else queued on it,
so it streams gap-free; the scan chain runs entirely off-spine (PSUM->SBUF
copies and 1/sqrt on ACT, init terms accumulated on the PE, f-recurrence
small ops on GPSIMD, transposes on the PE).  Early samples normalize on
ACT/GPSIMD while the spine runs; the whole tail (t>=16) is ONE fused scan
group whose normalizes stream on DVE right after the spine drains, with
their output DMAs on the SP queue (a GPSIMD teardown drain would stall the
SWDGE queue).  The tail group also skips bn_aggr entirely -- its combine
matmuls consume the raw bn_stats fields (means/cnt*var) with 1/16 and
1/4096 masks accumulated in PSUM -- and its recurrence small-ops run on the
now-idle DVE instead of GPSIMD, shortening the one off-spine chain.

Sharding: channels C=256 split across 8 cores (32 each) -- every channel's
recurrence is independent. Per core the shard sits resident in SBUF as
[128 partitions, 32 t, 1024 f] bf16, partition p = q*32 + c (q = one of 4
spatial blocks, c = channel).
"""

import os
import sys

import numpy as np

sys.path.insert(0, "/opt/trn_rl_repo")

B = 32          # batch (sequential scan axis)
H = 64
W_SP = 64
C = 256
NCORES = 8
CS = C // NCORES    # 32 channels per core
Q = 4               # spatial blocks per sample
F = (H * W_SP) // Q  # 1024 elements per block
P = 128             # partitions (Q*CS)
AFWD = 0.999
EPS = 1e-5
# scan groups (samples per scan matmul batch); tapered head for early output
GROUPS = [2, 6, 8, 16]
assert sum(GROUPS) == B

# engine assignment by sample index.  ACT_STAT samples get their stats from
# two ACT accumulate passes (Identity -> sum, Square -> sumsq) instead of DVE
# bn_stats, offloading the DVE stat stream; they sit at the tails of groups
# 1-3 so the ACT work overlaps DVE's bn_stats of the same group's head.
ACT_STAT = set()                            # pure bn_stats spine on DVE
NORM_GPS = {3, 7, 11, 15}                   # one GPSIMD norm per odd pair
NORM_DVE = set(range(16, 32))               # tail norms: DVE epilogue after spine

# const blob free-axis layout (f32, [128, CBLOB])
OFF_W = 0            # w     [B, B]     rows 0..31
OFF_MASK = OFF_W + B      # mask  [P, CS]
OFF_BMASK = OFF_MASK + CS  # bmask [CS, P]  rows 0..31
OFF_INIT = OFF_BMASK + P  # init  [CS, 2B] rows 0..31
OFF_EYE = OFF_INIT + 2 * B  # eye   [CS, CS] rows 0..31
CBLOB = OFF_EYE + CS

LAST_EXEC_NS = None
LAST_RESULTS = None
_COMPILED = {}


def _ensure_ntff_hook():
    """The axon boot degrades silently when ``antenv.axon_hooks`` is missing;
    provide the module + the ctypes-based NRT-profile hook ourselves so
    ``run_bass_kernel_spmd(trace=True)`` can capture NTFF profiles."""
    try:
        from antenv.axon_hooks import get_axon_ntff_profile_hook  # noqa: F401

        return
    except ImportError:
        pass

    import contextlib
    import ctypes
    import types

    so_path = "/opt/axon/libaxon_pjrt.so"
    state = {"hook": None}

    mod = types.ModuleType("antenv.axon_hooks")

    def set_axon_ntff_profile_hook(h):
        state["hook"] = h

    def get_axon_ntff_profile_hook():
        return state["hook"]

    mod.set_axon_ntff_profile_hook = set_axon_ntff_profile_hook
    mod.get_axon_ntff_profile_hook = get_axon_ntff_profile_hook
    import antenv

    antenv.axon_hooks = mod
    sys.modules["antenv.axon_hooks"] = mod

    if not os.path.exists(so_path):
        return
    lib = ctypes.CDLL(so_path)
    if not hasattr(lib, "axon_start_nrt_profile"):
        return
    lib.axon_start_nrt_profile.argtypes = [
        ctypes.POINTER(ctypes.c_int64),
        ctypes.c_size_t,
    ]
    lib.axon_start_nrt_profile.restype = ctypes.c_int64
    lib.axon_stop_nrt_profile.argtypes = [ctypes.c_char_p]
    lib.axon_stop_nrt_profile.restype = ctypes.c_int64

    @contextlib.contextmanager
    def _hook(output_dir, device_ids):
        import jax

        jax.devices()
        if device_ids:
            ids = (ctypes.c_int64 * len(device_ids))(*device_ids)
            rc = lib.axon_start_nrt_profile(ids, len(device_ids))
        else:
            rc = lib.axon_start_nrt_profile(None, 0)
        if rc != 0:
            raise RuntimeError(f"axon_start_nrt_profile rc={rc}")
        try:
            yield
        finally:
            n = lib.axon_stop_nrt_profile(str(output_dir).encode())
            print(f"profile: {n} file(s) written to {output_dir}", file=sys.stderr)

    state["hook"] = _hook


def _build_bass():
    from contextlib import ExitStack

    import concourse.bacc as bacc
    import concourse.tile as tile
    from concourse import mybir

    DT = mybir.dt.float32
    BF = mybir.dt.bfloat16
    Alu = mybir.AluOpType
    Act = mybir.ActivationFunctionType
    Ax = mybir.AxisListType

    nc = bacc.Bacc(
        "TRN2", target_bir_lowering=False, debug=False, num_devices=NCORES
    )
    x_h = nc.declare_dram_parameter("x", [P, B, F], BF, isOutput=False)
    blob_h = nc.declare_dram_parameter("blob", [P, CBLOB], DT, isOutput=False)
    maskb_h = nc.declare_dram_parameter("maskb", [P, 3 * CS], BF, isOutput=False)
    out_h = nc.declare_dram_parameter("out", [P, B, F], BF, isOutput=True)

    LMAX = max(GROUPS)
    NPAIR = B // 2

    with tile.TileContext(nc) as tc, ExitStack() as ctx:
        consts = ctx.enter_context(tc.tile_pool(name="consts", bufs=1))
        xpool = ctx.enter_context(tc.tile_pool(name="xp", bufs=1))
        sqpool = ctx.enter_context(tc.tile_pool(name="sqp", bufs=3))
        small = ctx.enter_context(tc.tile_pool(name="small", bufs=1))
        gpool = ctx.enter_context(tc.tile_pool(name="gp", bufs=3))
        psum = ctx.enter_context(tc.tile_pool(name="ps", bufs=2, space="PSUM"))

        blob = consts.tile([P, CBLOB], DT)
        sb_maskb3 = consts.tile([P, 3 * CS], BF)  # /Q | /16 | /4096 in bf16
        sb_maskb = sb_maskb3[:, 0:CS]
        sb_mask16 = sb_maskb3[:, CS : 2 * CS]
        sb_mask4k = sb_maskb3[:, 2 * CS : 3 * CS]
        sb_w = blob[0:B, OFF_W : OFF_W + B]          # [B, B]
        sb_mask = blob[:, OFF_MASK : OFF_MASK + CS]  # [P, CS]  (p%CS==c)/(Q*F)
        sb_bmask = blob[0:CS, OFF_BMASK : OFF_BMASK + P]  # [CS, P]
        sb_init = blob[0:CS, OFF_INIT : OFF_INIT + 2 * B]  # [CS, 2B]
        sb_eye = blob[0:CS, OFF_EYE : OFF_EYE + CS]  # [CS, CS] identity

        xbig = xpool.tile([P, B, F], BF)        # resident shard, 64 KiB/partition
        # ---- all input DMAs up front (SP queue); sample 0 triggers before
        # the const blobs (consts aren't needed until the first scan ~+6us)
        nc.sync.dma_start(out=xbig[:, 0, 0 : F // 2], in_=x_h[:, 0, 0 : F // 2])
        nc.sync.dma_start(out=xbig[:, 0, F // 2 : F], in_=x_h[:, 0, F // 2 : F])
        nc.sync.dma_start(out=blob, in_=blob_h[:, :])
        nc.sync.dma_start(out=sb_maskb3, in_=maskb_h[:, :])
        nc.sync.dma_start(out=xbig[:, 1:2, :], in_=x_h[:, 1:2, :])
        # singles for t2..5 -- finer landing granularity while the DMA
        # engines ramp, so the bn spine waits less for early samples
        for t in range(2, 10):
            nc.sync.dma_start(out=xbig[:, t : t + 1, :], in_=x_h[:, t : t + 1, :])
        for k in range(5, NPAIR):
            nc.sync.dma_start(
                out=xbig[:, 2 * k : 2 * k + 2, :], in_=x_h[:, 2 * k : 2 * k + 2, :]
            )

        sb_eps = consts.tile([CS, 1], DT)
        nc.vector.memset(sb_eps, EPS)

        st6 = small.tile([P, B, 2, 6], BF)      # bn_stats out, 2 chunks/sample
        mv = small.tile([P, 2, B], BF)          # [,0,t]=mean  [,1,t]=var (pre-fix)
        s2 = small.tile([P, 2, B], DT)          # ACT_STAT raw sums / sumsq
        # Scan state in ct layout ([channel, t]): per-group writes slice the
        # FREE axis (partition slices must start at 0 on compute engines).
        mu_msq = small.tile([CS, 2 * B], DT)    # cols t: mu_ct; cols B+t: msq_ct
        mu_msq3 = mu_msq.rearrange("p (two b) -> p two b", two=2)
        mu_tc = small.tile([B, CS], DT)         # transpose scratch for the scans
        f_ct = small.tile([CS, B], DT)          # f = var + a*d^2
        f_tc = small.tile([B, CS], DT)
        rb = small.tile([P, 2 * B], DT)         # rb[p, t]=rscale; rb[p, B+t]=nbias
        rb3 = rb.rearrange("p (two b) -> p two b", two=2)
        nc.vector.memset(mu_msq, 0.0)
        nc.vector.memset(f_ct, 0.0)

        out_q = [0]

        def stats(t0, L):
            for t in range(t0, t0 + L):
                if t in ACT_STAT:
                    sq = sqpool.tile([P, F], BF)
                    nc.scalar.activation(
                        out=sq, in_=xbig[:, t, :], func=Act.Identity,
                        accum_out=s2[:, 0, t : t + 1],
                    )
                    sq2 = sqpool.tile([P, F], BF)
                    nc.scalar.activation(
                        out=sq2, in_=xbig[:, t, :], func=Act.Square,
                        accum_out=s2[:, 1, t : t + 1],
                    )
                else:
                    # mean+var per partition-block in ONE DVE pass (bn_stats)
                    nc.vector.bn_stats(
                        out=st6[:, t, 0, :], in_=xbig[:, t, 0 : F // 2]
                    )
                    nc.vector.bn_stats(
                        out=st6[:, t, 1, :], in_=xbig[:, t, F // 2 : F]
                    )


        def scan_and_norm(t0, L):
            cols = slice(t0, t0 + L)
            vcols = slice(B + t0, B + t0 + L)

            ps_stats = psum.tile([CS, 2, LMAX, 2, 2], DT, tag="ps_stats")
            if t0 >= 16:
                # tail: combine q-blocks AND bn chunks straight from raw
                # bn_stats fields (means at [...,1,4], cnt*var at [...,2,5]):
                # mu  = (1/16)  sum mean_field       (4q x 2chunk x 2eo)
                # msq = (1/4096) sum cv + (1/16) sum mean^2
                st6r = st6.rearrange("p b c (k three) -> p b c k three", three=3)
                means = st6r[:, cols, :, :, 1]      # [P, L, 2, 2]
                cvs = st6r[:, cols, :, :, 2]
                m2c = gpool.tile([P, LMAX, 2, 2], BF, tag="m2c")
                nc.vector.tensor_mul(out=m2c[:, 0:L, :, :], in0=means, in1=means)
                nc.tensor.matmul(
                    out=ps_stats[:, 0, 0:L, :, :], lhsT=sb_mask16, rhs=means,
                    start=True, stop=True,
                )
                nc.tensor.matmul(
                    out=ps_stats[:, 1, 0:L, :, :], lhsT=sb_mask4k, rhs=cvs,
                    start=True, stop=False,
                )
                nc.tensor.matmul(
                    out=ps_stats[:, 1, 0:L, :, :], lhsT=sb_mask16,
                    rhs=m2c[:, 0:L, :, :], start=False, stop=True,
                )
                ps_c4 = ps_stats.rearrange("p two l a b -> p two l (a b)")
                nc.vector.tensor_reduce(
                    out=mu_msq3[:, :, cols], in_=ps_c4[:, :, 0:L, :],
                    axis=Ax.X, op=Alu.add,
                )
            else:
                # early groups: combine raw bn_stats fields without touching
                # the DVE spine.  The 4 per-sample (chunk x even/odd) field
                # values sit at uniform stride 3; GPSIMD pre-sums them, PE
                # combines q-blocks with 3 accumulated matmuls.
                st6v = st6.rearrange("p b c (k three) -> p b (c k) three", three=3)
                ms = gpool.tile([P, 4, LMAX], BF, tag="ms")
                # ms[:,0]=sum means; ms[:,1]=sum cv; ms[:,2]=sum mean^2
                nc.gpsimd.tensor_add(
                    out=ms[:, 3, 0:L], in0=st6v[:, cols, 0, 1],
                    in1=st6v[:, cols, 1, 1],
                )
                nc.gpsimd.tensor_add(
                    out=ms[:, 0, 0:L], in0=st6v[:, cols, 2, 1],
                    in1=st6v[:, cols, 3, 1],
                )
                nc.gpsimd.tensor_add(
                    out=ms[:, 0, 0:L], in0=ms[:, 0, 0:L], in1=ms[:, 3, 0:L]
                )
                nc.gpsimd.tensor_add(
                    out=ms[:, 3, 0:L], in0=st6v[:, cols, 0, 2],
                    in1=st6v[:, cols, 1, 2],
                )
                nc.gpsimd.tensor_add(
                    out=ms[:, 1, 0:L], in0=st6v[:, cols, 2, 2],
                    in1=st6v[:, cols, 3, 2],
                )
                nc.gpsimd.tensor_add(
                    out=ms[:, 1, 0:L], in0=ms[:, 1, 0:L], in1=ms[:, 3, 0:L]
                )
                m2q = gpool.tile([P, 4, LMAX], BF, tag="m2q")
                for j in range(4):
                    nc.gpsimd.tensor_mul(
                        out=m2q[:, j, 0:L], in0=st6v[:, cols, j, 1],
                        in1=st6v[:, cols, j, 1],
                    )
                nc.gpsimd.tensor_add(
                    out=m2q[:, 0, 0:L], in0=m2q[:, 0, 0:L], in1=m2q[:, 1, 0:L]
                )
                nc.gpsimd.tensor_add(
                    out=m2q[:, 2, 0:L], in0=m2q[:, 2, 0:L], in1=m2q[:, 3, 0:L]
                )
                nc.gpsimd.tensor_add(
                    out=ms[:, 2, 0:L], in0=m2q[:, 0, 0:L], in1=m2q[:, 2, 0:L]
                )
                nc.tensor.matmul(
                    out=ps_stats[:, 0, 0:L, 0, 0], lhsT=sb_mask16,
                    rhs=ms[:, 0, 0:L], start=True, stop=True,
                )
                nc.tensor.matmul(
                    out=ps_stats[:, 1, 0:L, 0, 0], lhsT=sb_mask4k,
                    rhs=ms[:, 1, 0:L], start=True, stop=False,
                )
                nc.tensor.matmul(
                    out=ps_stats[:, 1, 0:L, 0, 0], lhsT=sb_mask16,
                    rhs=ms[:, 2, 0:L], start=False, stop=True,
                )
                nc.scalar.copy(
                    out=mu_msq3[:, :, cols], in_=ps_stats[:, :, 0:L, 0, 0]
                )

            # ---- s_mu_{t-1} for this group's t-range, directly in ct ----
            # out[c,t] = sum_i mu_tc[i,c] * w[i,t]; cols beyond the prefix are
            # zeros and W kills rows >= t anyway
            ps_mt = psum.tile([B, CS], DT, tag="ps_t")
            nc.tensor.transpose(out=ps_mt, in_=mu_msq[:, 0:B], identity=sb_eye)
            nc.scalar.copy(out=mu_tc, in_=ps_mt)
            ps_smu = psum.tile([CS, LMAX], DT, tag="ps_sv")
            nc.tensor.matmul(
                out=ps_smu[:, 0:L], lhsT=mu_tc, rhs=sb_w[:, cols],
                start=True, stop=False,
            )
            nc.tensor.matmul(
                out=ps_smu[:, 0:L], lhsT=sb_eye, rhs=sb_init[:, cols],
                start=False, stop=True,
            )
            smu_g = gpool.tile([CS, LMAX], DT, tag="smu_g")
            nc.scalar.copy(out=smu_g[:, 0:L], in_=ps_smu[:, 0:L])

            # ---- f = (msq - mu^2) + a*(mu - smu)^2  (all [CS, L], ct) ----
            ve = nc.vector if t0 >= 16 else nc.gpsimd
            mu_cols = mu_msq[:, cols]
            m2 = gpool.tile([CS, LMAX], DT, tag="m2")
            ve.tensor_mul(out=m2[:, 0:L], in0=mu_cols, in1=mu_cols)
            var_g = gpool.tile([CS, LMAX], DT, tag="var_g")
            ve.tensor_sub(
                out=var_g[:, 0:L], in0=mu_msq[:, vcols], in1=m2[:, 0:L]
            )
            d_g = gpool.tile([CS, LMAX], DT, tag="d_g")
            ve.tensor_sub(out=d_g[:, 0:L], in0=mu_cols, in1=smu_g[:, 0:L])
            d2_g = gpool.tile([CS, LMAX], DT, tag="d2_g")
            ve.tensor_mul(out=d2_g[:, 0:L], in0=d_g[:, 0:L], in1=d_g[:, 0:L])
            if ve is nc.vector:
                ve.scalar_tensor_tensor(
                    out=f_ct[:, cols],
                    in0=d2_g[:, 0:L],
                    scalar=AFWD,
                    in1=var_g[:, 0:L],
                    op0=Alu.mult,
                    op1=Alu.add,
                )
            else:
                # gpsimd has no scalar_tensor_tensor: fold a into d2, then add
                ve.tensor_scalar(
                    out=d2_g[:, 0:L], in0=d2_g[:, 0:L],
                    scalar1=AFWD, scalar2=None, op0=Alu.mult,
                )
                ve.tensor_add(
                    out=f_ct[:, cols], in0=d2_g[:, 0:L], in1=var_g[:, 0:L]
                )

            # ---- s_var_{t-1} via the same W contraction on f, direct ct ----
            ps_ft = psum.tile([B, CS], DT, tag="ps_t")
            nc.tensor.transpose(out=ps_ft, in_=f_ct, identity=sb_eye)
            nc.scalar.copy(out=f_tc, in_=ps_ft)
            ps_svar = psum.tile([CS, LMAX], DT, tag="ps_sv")
            nc.tensor.matmul(
                out=ps_svar[:, 0:L], lhsT=f_tc, rhs=sb_w[:, cols],
                start=True, stop=False,
            )
            nc.tensor.matmul(
                out=ps_svar[:, 0:L], lhsT=sb_eye, rhs=sb_init[:, vcols],
                start=False, stop=True,
            )

            # ---- rscale = 1/sqrt(svar+eps) in ONE ACT op, from PSUM ----
            rs_g = gpool.tile([CS, LMAX], DT, tag="rs_g")
            nc.scalar.activation(
                out=rs_g[:, 0:L],
                in_=ps_svar[:, 0:L],
                func=Act.Abs_reciprocal_sqrt,
                bias=sb_eps,
                scale=1.0,
            )
            nb_g = gpool.tile([CS, LMAX], DT, tag="nb_g")
            if ve is nc.vector:
                ve.scalar_tensor_tensor(
                    out=nb_g[:, 0:L],
                    in0=smu_g[:, 0:L],
                    scalar=-1.0,
                    in1=rs_g[:, 0:L],
                    op0=Alu.mult,
                    op1=Alu.mult,
                )
            else:
                ve.tensor_scalar(
                    out=nb_g[:, 0:L], in0=smu_g[:, 0:L],
                    scalar1=-1.0, scalar2=None, op0=Alu.mult,
                )
                ve.tensor_mul(
                    out=nb_g[:, 0:L], in0=nb_g[:, 0:L], in1=rs_g[:, 0:L]
                )

            # ---- broadcast to all 128 partitions via PE ----
            ps_rb = psum.tile([P, 2, LMAX], DT, tag="ps_rb")
            nc.tensor.matmul(
                out=ps_rb[:, 0, 0:L],
                lhsT=sb_bmask,
                rhs=rs_g[:, 0:L],
                start=True,
                stop=True,
            )
            nc.tensor.matmul(
                out=ps_rb[:, 1, 0:L],
                lhsT=sb_bmask,
                rhs=nb_g[:, 0:L],
                start=True,
                stop=True,
            )
            nc.scalar.copy(out=rb3[:, :, cols], in_=ps_rb[:, :, 0:L])

            # ---- normalize in place + stream out per pair ----
            for t in range(t0, t0 + L):
                if t in NORM_DVE:
                    continue      # deferred to the DVE epilogue after the spine
                if t in NORM_GPS:
                    nc.gpsimd.tensor_scalar(
                        out=xbig[:, t, :],
                        in0=xbig[:, t, :],
                        scalar1=rb[:, t : t + 1],
                        scalar2=rb[:, B + t : B + t + 1],
                        op0=Alu.mult,
                        op1=Alu.add,
                    )
                else:
                    nc.scalar.activation(
                        out=xbig[:, t, :],
                        in_=xbig[:, t, :],
                        func=Act.Identity,
                        bias=rb[:, B + t : B + t + 1],
                        scale=rb[:, t : t + 1],
                    )
                if t % 2 == 1:
                    pr = slice(t - 1, t + 1)
                    # alternate SWDGE (gpsimd) / HWDGE (sync) trigger queues
                    eng = nc.gpsimd if out_q[0] % 2 == 0 else nc.sync
                    eng.dma_start(out=out_h[:, pr, :], in_=xbig[:, pr, :])
                    out_q[0] += 1

        # software-pipelined: stats run two groups ahead of scan+norm
        spans = []
        t0 = 0
        for L in GROUPS:
            spans.append((t0, L))
            t0 += L
        DEPTH = 2
        for i, (s0, sl) in enumerate(spans):
            stats(s0, sl)
            if i >= DEPTH:
                scan_and_norm(*spans[i - DEPTH])
        for i in range(len(spans) - DEPTH, len(spans)):
            scan_and_norm(*spans[i])

        # ---- DVE norm epilogue: rb for these groups is long ready, so the
        # bn spine never stalls and these stream back-to-back at 2x rate ----
        for t in sorted(NORM_DVE):
            nc.vector.tensor_scalar(
                out=xbig[:, t, :],
                in0=xbig[:, t, :],
                scalar1=rb[:, t : t + 1],
                scalar2=rb[:, B + t : B + t + 1],
                op0=Alu.mult,
                op1=Alu.add,
            )
            if t % 2 == 1:
                pr = slice(t - 1, t + 1)
                # sync only: a GPSIMD drain at teardown can stall its queue
                nc.sync.dma_start(out=out_h[:, pr, :], in_=xbig[:, pr, :])

    nc.compile()
    return nc


def _blob():
    i = np.arange(B)[:, None].astype(np.float64)
    t = np.arange(B)[None, :].astype(np.float64)
    w = np.where(i < t, (1.0 - AFWD) * AFWD ** (t - 1.0 - i), 0.0).astype(np.float32)
    mask = np.zeros((P, CS), np.float32)
    mask[np.arange(P), np.arange(P) % CS] = 1.0 / Q
    bmask = np.zeros((CS, P), np.float32)
    bmask[np.arange(P) % CS, np.arange(P)] = 1.0
    blob = np.zeros((P, CBLOB), np.float32)
    blob[0:B, OFF_W : OFF_W + B] = w
    blob[:, OFF_MASK : OFF_MASK + CS] = mask
    blob[0:CS, OFF_BMASK : OFF_BMASK + P] = bmask
    blob[0:CS, OFF_EYE : OFF_EYE + CS] = np.eye(CS, dtype=np.float32)
    return blob


def kernel(**inputs):
    global LAST_EXEC_NS, LAST_RESULTS
    import ml_dtypes

    x = np.asarray(inputs["x"], dtype=np.float32)
    mu0 = np.asarray(inputs["mu0"], dtype=np.float32)
    var0 = np.asarray(inputs["var0"], dtype=np.float32)
    assert x.shape == (B, H, W_SP, C)

    from concourse.bass_utils import run_bass_kernel_spmd

    if "nc" not in _COMPILED:
        _COMPILED["nc"] = _build_bass()
    nc = _COMPILED["nc"]

    apow = (AFWD ** np.arange(B, dtype=np.float64)).astype(np.float32)[None, :]

    # [B, Q, F, C] view of x; per-core shard is [Q, CS, B, F] -> [P, B, F]
    x16 = x.astype(ml_dtypes.bfloat16)
    xr = x16.reshape(B, Q, F, C)
    in_maps = []
    for core in range(NCORES):
        c0 = core * CS
        xs = np.ascontiguousarray(
            xr[:, :, :, c0 : c0 + CS].transpose(1, 3, 0, 2)
        ).reshape(P, B, F)
        blob = _blob()
        blob[0:CS, OFF_INIT : OFF_INIT + B] = mu0[c0 : c0 + CS, None] * apow
        blob[0:CS, OFF_INIT + B : OFF_INIT + 2 * B] = (
            var0[c0 : c0 + CS, None] * apow
        )
        import ml_dtypes as _md
        maskb = np.zeros((P, 3 * CS), np.float32)
        maskb[np.arange(P), np.arange(P) % CS] = 1.0 / Q
        maskb[np.arange(P), CS + np.arange(P) % CS] = 1.0 / 16.0
        maskb[np.arange(P), 2 * CS + np.arange(P) % CS] = 1.0 / 4096.0
        in_maps.append(
            {"x": xs, "blob": blob, "maskb": maskb.astype(_md.bfloat16)}
        )

    trace = bool(int(os.environ.get("NORM_KERNEL_TRACE", "0")))
    if trace:
        _ensure_ntff_hook()
    res = run_bass_kernel_spmd(nc, in_maps, list(range(NCORES)), trace=trace)
    LAST_EXEC_NS = res.exec_time_ns
    LAST_RESULTS = res

    out = np.empty((B, Q, F, C), np.float32)
    for core in range(NCORES):
        c0 = core * CS
        o = np.asarray(res.results[core]["out"]).astype(np.float32)
        o = o.reshape(Q, CS, B, F)
        out[:, :, :, c0 : c0 + CS] = o.transpose(2, 0, 3, 1)
    return out.reshape(B, H, W_SP, C)
